# revision 14
# baseline (speedup 1.0000x reference)
"""2D DCT-II (ortho) on (32, 3, 512, 512) fp32, data-parallel across 8 TRN2 NeuronCores.

The DCT along an axis is a matmul with the constant 512x512 DCT matrix D:
    out = D @ X @ D.T
Structure (per 512x512 image, all matmuls float32r, fp32 PSUM accumulation):
  0. W-axis even/odd fold (DVE, reversed-AP second operand):
       EW[n,w'] = X[n,w'] + X[n,511-w'],  OW[n,w'] = X[n,w'] - X[n,511-w']
     (uses D[k, 511-w] = (-1)^k D[k, w], halving pass B's contraction)
  1. Pass A (H-DCT): P1e = EW.T @ D.T, P1o = OW.T @ D.T  via lhsT=EW/OW
     chunks (data stationary), rhs = D.T. 16 matmuls N=512.
  2. Pass B (W-DCT): OUT[:, 2j] = P1e.T @ DeW.T, OUT[:, 2j+1] = P1o.T @ DoW.T
     with DeW[j,w'] = D[2j,w'], DoW[j,w'] = D[2j+1,w']. 16 matmuls N=256;
     the PSUM->SBUF copy interleaves the even/odd column blocks.
Engine budget per image: PE 32 MMs (~6us), DVE folds + pass-B interleave
copies, ACT pass-A copies + store dispatch, sync load dispatch.
"""
import os
import sys

for _p in ("/opt/trn_rl_repo", os.path.expanduser("~/.axon_site/_ro/trn_rl_repo")):
    if os.path.isdir(_p) and _p not in sys.path:
        sys.path.insert(0, _p)

import numpy as np
import concourse.bass as bass
import concourse.bacc as bacc
import concourse.mybir as mybir
import concourse.tile as tile
from concourse.bass_utils import run_bass_kernel_spmd

dt = mybir.dt

N = 512            # image height/width
H = N // 2         # 256, folded width
P = 128            # SBUF partitions
C = N // P         # 4 row-chunks per image
N_CORES = 8
B, CH = 32, 3      # full input batch/channels
IMGS = (B * CH) // N_CORES  # 12 images per core


def _dct_matrix() -> np.ndarray:
    n = np.arange(N, dtype=np.float64)
    k = n[:, None]
    D = np.cos(np.pi * (2.0 * n[None, :] + 1.0) * k / (2.0 * N))
    D[0] *= np.sqrt(1.0 / N)
    D[1:] *= np.sqrt(2.0 / N)
    return D


def _consts() -> tuple[np.ndarray, np.ndarray]:
    D = _dct_matrix()
    dct_t = np.ascontiguousarray(D.T.astype(np.float32))            # [n, k]
    de_t = np.ascontiguousarray(D[0::2, :H].T.astype(np.float32))   # [256, 256]
    do_t = np.ascontiguousarray(D[1::2, :H].T.astype(np.float32))   # [256, 256]
    deo = np.concatenate([de_t, do_t], axis=0)                      # [512, 256]
    return dct_t, deo


def _build_nc() -> bacc.Bacc:
    nc = bacc.Bacc("TRN2", target_bir_lowering=False, debug=False, num_devices=N_CORES)
    inp = nc.dram_tensor("inp", [IMGS, N, N], dt.float32r, kind="ExternalInput")
    out = nc.dram_tensor("out", [IMGS, N, N], dt.float32, kind="ExternalOutput")
    dct_t = nc.dram_tensor("dct_t", [N, N], dt.float32r, kind="ExternalInput")
    deo_t = nc.dram_tensor("deo_t", [N, H], dt.float32r, kind="ExternalInput")

    f32r = dt.float32r
    f32 = dt.float32

    with tile.TileContext(nc) as tc:
        with (
            tc.tile_pool(name="const", bufs=1) as const_pool,
            tc.tile_pool(name="xin", bufs=4) as xin_pool,
            tc.tile_pool(name="eo", bufs=2) as eo_pool,
            tc.tile_pool(name="mid", bufs=2) as mid_pool,
            tc.tile_pool(name="res", bufs=2) as res_pool,
            tc.tile_pool(name="ps", bufs=3, space="PSUM") as psa_pool,
            tc.tile_pool(name="psb", bufs=2, space="PSUM") as psb_pool,
        ):
            # D.T resident in SBUF: dt_sb[p, 512*c + f] = D.T[128*c + p, f]
            dt_sb = const_pool.tile([P, C * N], f32r)
            nc.scalar.dma_start(
                dt_sb[:].rearrange("p (c f) -> p c f", c=C),
                dct_t.ap().rearrange("(c p) f -> p c f", p=P),
            )
            # deo_sb[p, 256*q + j] = deo[128*q + p, j]; q=0,1 even, q=2,3 odd
            deo_sb = const_pool.tile([P, C * H], f32r)
            nc.scalar.dma_start(
                deo_sb[:].rearrange("p (q j) -> p q j", q=C),
                deo_t.ap().rearrange("(q p) j -> p q j", p=P),
            )

            # PE warmup during the initial DMA ramp: ~10 dummy matmuls flip the
            # HAM clock gate to 8/8 before the first real matmul arrives.
            scr_f = const_pool.tile([P, N + P], f32)
            nc.gpsimd.memset(scr_f[:], 0.0)
            scr = const_pool.tile([P, N + P], f32r)
            nc.vector.tensor_copy(scr[:], scr_f[:])
            ps_w = psb_pool.tile([P, N], f32, tag="psB")
            for _ in range(10):
                nc.tensor.matmul(
                    ps_w[:], scr[:, N : N + P], scr[:, :N], start=True, stop=True
                )

            for i in range(IMGS):
                # x_sb[p, 512*c + w] = X[128*c + p, w]
                x_sb = xin_pool.tile([P, C * N], f32r, tag="x")
                nhalf = 2 if i == 0 else 1  # finer pipelining for the first image
                for hh in range(nhalf):
                    cs, ce = hh * C // nhalf, (hh + 1) * C // nhalf
                    nc.sync.dma_start(
                        x_sb[:, N * cs : N * ce].rearrange("p (c f) -> p c f", c=ce - cs),
                        inp.ap()[i][P * cs : P * ce, :].rearrange("(c p) f -> p c f", p=P),
                    )

                # W fold; for image 0 split per half for earlier matmul start,
                # emitting the adds first (pass A consumes EW windows first)
                eo_sb = eo_pool.tile([P, 2 * C * H], f32r, tag="eo")
                xa = x_sb[:]
                for par in range(2):  # 0: add -> EW, 1: sub -> OW
                    for hh in range(nhalf):
                        cs, ce = hh * C // nhalf, (hh + 1) * C // nhalf
                        nc_ = ce - cs
                        lo = bass.AP(
                            xa.tensor, xa.offset + N * cs,
                            [[xa.ap[0][0], P], [N, nc_], [1, H]],
                        )
                        hi_rev = bass.AP(
                            xa.tensor, xa.offset + N * cs + N - 1,
                            [[xa.ap[0][0], P], [N, nc_], [-1, H]],
                        )
                        dst = eo_sb[
                            :, par * C * H + H * cs : par * C * H + H * ce
                        ].rearrange("p (c j) -> p c j", c=nc_)
                        if par == 0:
                            nc.vector.tensor_add(dst, lo, hi_rev)
                        else:
                            nc.vector.tensor_sub(dst, lo, hi_rev)

                # pass A (H-DCT): t in {e0,e1,o0,o1}; t-pairs share a 2-bank psum
                p1_sb = mid_pool.tile([P, C * N], f32r, tag="p1")
                for tp in range(2):
                    ps = psa_pool.tile([P, 2 * N], f32, tag="psA")
                    for t2 in range(2):
                        eo_base = tp * C * H + t2 * P
                        for c in range(C):
                            lhsT = eo_sb[:, eo_base + H * c : eo_base + H * c + P]
                            rhs = dt_sb[:, N * c : N * (c + 1)]
                            nc.tensor.matmul(
                                ps[:, N * t2 : N * (t2 + 1)], lhsT, rhs,
                                start=(c == 0), stop=(c == C - 1),
                            )
                    nc.scalar.copy(p1_sb[:, 2 * N * tp : 2 * N * (tp + 1)], ps[:])

                # pass B (W-DCT): k_h windows m, single-bank psums
                o_sb = res_pool.tile([P, C * N], f32, tag="o")
                for m in range(C):
                    ps = psb_pool.tile([P, N], f32, tag="psB")
                    for half in range(2):  # 0: even k_w, 1: odd k_w
                        for c2 in range(2):
                            q = 2 * half + c2
                            lhsT = p1_sb[:, N * q + P * m : N * q + P * (m + 1)]
                            rhs = deo_sb[:, H * q : H * (q + 1)]
                            nc.tensor.matmul(
                                ps[:, H * half : H * (half + 1)], lhsT, rhs,
                                start=(c2 == 0), stop=(c2 == 1),
                            )
                    # interleave: o_sb[p, 512m + 2j + h] = ps[p, 256*h + j]
                    src = ps[:].rearrange("p (h j) -> p h j", h=2)
                    ob = o_sb[:]
                    dst = bass.AP(
                        ob.tensor, ob.offset + N * m,
                        [[ob.ap[0][0], P], [1, 2], [2, H]],
                    )
                    nc.vector.tensor_copy(dst, src)
                    if m % 2 == 1:  # store half-image once its windows landed
                        mp = m // 2
                        nc.scalar.dma_start(
                            out.ap()[i][2 * P * mp : 2 * P * (mp + 1), :].rearrange(
                                "(c p) f -> p c f", p=P
                            ),
                            o_sb[:, 2 * N * mp : 2 * N * (mp + 1)].rearrange(
                                "p (c f) -> p c f", c=2
                            ),
                        )

    nc.compile()
    return nc


_NC_CACHE: bacc.Bacc | None = None


def _get_nc() -> bacc.Bacc:
    global _NC_CACHE
    if _NC_CACHE is None:
        _NC_CACHE = _build_nc()
    return _NC_CACHE


def run(inp: np.ndarray, **spmd_kwargs):
    """Shard, run on 8 cores, gather. Returns (output, BassKernelResults)."""
    x = np.asarray(inp, dtype=np.float32)
    assert x.shape == (B, CH, N, N), x.shape
    shards = x.reshape(N_CORES, IMGS, N, N)
    dct_t, deo = _consts()
    in_maps = [
        {"inp": np.ascontiguousarray(shards[c]), "dct_t": dct_t, "deo_t": deo}
        for c in range(N_CORES)
    ]
    res = run_bass_kernel_spmd(_get_nc(), in_maps, core_ids=list(range(N_CORES)), **spmd_kwargs)
    out = np.stack([res.results[c]["out"] for c in range(N_CORES)])
    return out.reshape(B, CH, N, N), res


def kernel(inp: np.ndarray) -> np.ndarray:
    out, _ = run(inp)
    return out
